# revision 17
# baseline (speedup 1.0000x reference)
"""Trainium2 Bass kernel for nn_BiLSTM_50500225466406.

2-layer BiLSTM (H=200) over word embeddings (E=300), B=32, S=128, + sigmoid
linear head (17 tags).  Char-CNN branch in the reference is dead code.

Strategy (8 NeuronCores):
  - Data-parallel over batch: 4 sentences per core, zero cross-core comms.
  - Embedding gather on host (X^T per core is a kernel input, bf16).
  - All matmul operands bf16 (single-pass PE; fp32 would run LOW_HIGH
    2-pass).  PSUM accumulation and all elementwise math stay fp32.
  - Per layer/direction: xw = X @ W precomputed on-device, stored as four
    [128, 800] tiles per direction with row r = (t%32)*4 + b.  Inside the
    recurrence each step's xw_t is injected into the gate PSUM tile with a
    single selector matmul per PSUM bank (lhsT = identity columns
    [:, (t%32)*4 : +4]), i.e. 2 matmuls instead of 7 chunked ones.
  - Recurrence keeps everything in [batch-partitions, gate-free] layout:
      * gates PSUM tile G [4, 1024] per direction: cols [i|f|g|o|pad|c~]
      * tanh-only activations: sigma(x) = (tanh(x/2)+1)/2.  i,f weight
        columns are pre-halved on host; o is halved by the ACT scale=0.5.
        States carried doubled: ct = 2c, ht = 2h; consumers' weights are
        pre-halved on host to compensate.
      * cell update with 3 scalar_tensor_tensor ops:
          A  = (tf + 1) * ct
          B  = (ti + 1) * tg
          ct' = 0.5*A + B            (= 2c')
        tanh(o/2) is a separate ACT op (off the h->h critical chain);
        tanh(ct'/2) is its own ACT op:
          ht = (to + 1) * tc         (= 2h)
      * ht transposed via PE matmul-with-identity into [H-part, time-col]
        accumulator tensors (bf16), which directly serve as next-step lhsT
        and as the next layer's (pre-transposed) input.
"""

import sys

for _p in ("/opt/trn_rl_repo",):
    if _p not in sys.path:
        sys.path.insert(0, _p)

import numpy as np
import ml_dtypes

import concourse.bass as bass
import concourse.mybir as mybir
import concourse.tile as tile
from concourse import bass_utils

F32 = mybir.dt.float32
BF16 = mybir.dt.bfloat16
NPBF16 = ml_dtypes.bfloat16
AF = mybir.ActivationFunctionType
ALU = mybir.AluOpType

# Problem constants (hardcoded per contract).
B, S, E, H, V, TAGS = 32, 128, 300, 200, 50002, 17
NCORES = 8
BL = B // NCORES          # 4 sentences per core
SB = S * BL               # 512 = time*batch columns per core
TBLK = 128 // BL          # 32 timesteps per xw tile
NJ = S // TBLK            # 4 xw tiles per direction
# G psum tile column layout (per direction):
#   [0:200] i  [200:400] f  [400:600] g  [600:800] o  [800:1000] ct state
GW = 1024
XK = (128, 128, 48)       # X^T partition chunks (300 emb dims + 1 ones + pad)
XROWS = 304
X1K = (128, 72, 128, 72, 1)   # layer-1 K chunks: fwd h(128,72), bwd h(128,72), ones
X1ROWS = 401
UK = (128, 72)            # recurrent K chunks of H=200


def _prep_weights(emb_table, lstm_Wih0, lstm_Whh0, lstm_b0,
                  lstm_Wih1, lstm_Whh1, lstm_b1, out_w, out_b):
    """Host-side weight transforms shared by all cores (cast to bf16)."""
    f32 = np.float32

    # layer0 input weights + bias: rows = 300 emb dims + ones row + pad
    w0 = np.zeros((XROWS, 1600), f32)
    for d in range(2):
        wt = lstm_Wih0[d].T.astype(f32).copy()        # [300, 800]
        wt[:, 0:400] *= 0.5                           # i,f halved
        b = lstm_b0[d].astype(f32).copy()
        b[0:400] *= 0.5
        w0[0:300, d * 800:(d + 1) * 800] = wt
        w0[300, d * 800:(d + 1) * 800] = b

    # layer0 recurrent weights: input is ht=2h -> *0.5 ; i,f further *0.5
    u0 = np.zeros((200, 1600), f32)
    for d in range(2):
        ut = lstm_Whh0[d].T.astype(f32) * 0.5
        ut = ut.copy()
        ut[:, 0:400] *= 0.5
        u0[:, d * 800:(d + 1) * 800] = ut

    # layer1 input weights: input is ht-concat (=2x) -> *0.5 ; i,f *0.5 ; bias plain
    w1 = np.zeros((X1ROWS, 1600), f32)
    for d in range(2):
        wt = lstm_Wih1[d].T.astype(f32) * 0.5         # [400, 800]
        wt = wt.copy()
        wt[:, 0:400] *= 0.5
        b = lstm_b1[d].astype(f32).copy()
        b[0:400] *= 0.5
        w1[0:400, d * 800:(d + 1) * 800] = wt
        w1[400, d * 800:(d + 1) * 800] = b

    u1 = np.zeros((200, 1600), f32)
    for d in range(2):
        ut = lstm_Whh1[d].T.astype(f32) * 0.5
        ut = ut.copy()
        ut[:, 0:400] *= 0.5
        u1[:, d * 800:(d + 1) * 800] = ut

    # output head: input is ht-concat -> *0.5 ; bias row plain
    ow = np.zeros((X1ROWS, TAGS), f32)
    ow[0:400, :] = out_w.T.astype(f32) * 0.5
    ow[400, :] = out_b.astype(f32)

    i128 = np.eye(128, dtype=f32)
    bf = NPBF16
    return {"w0": w0.astype(bf), "u0": u0.astype(bf), "w1": w1.astype(bf),
            "u1": u1.astype(bf), "ow": ow.astype(bf), "i128": i128.astype(bf)}


def _prep_xt(emb_table, words_shard):
    """[304, 512] X^T for one core: col n = emb[words[n%BL, n//BL]] + ones row."""
    idx = np.asarray(words_shard).T.reshape(-1)       # n = t*BL + b
    xt = np.zeros((XROWS, SB), np.float32)
    xt[0:300, :] = np.asarray(emb_table)[idx].T.astype(np.float32)
    xt[300, :] = 1.0
    return xt.astype(NPBF16)


# --------------------------------------------------------------------------
# Bass program
# --------------------------------------------------------------------------

def _emit_xw_precompute(nc, tc, psum_pool, w_chunks, rhs_chunks, XW, tag="xwp"):
    """XW[:, (d*NJ+j)*800 + n] = xw[t,b,n] with row (t%32)*4+b, tile j=t//32.

    w_chunks[k]: SBUF [Kp, 1600] bf16; rhs_chunks[k]: SBUF [Kp, SB] bf16
    (columns are tb = t*BL + b).  Output out[tb_chunk, units] = rhs.T @ w.
    """
    nk = len(w_chunks)
    for d in range(2):
        for j in range(NJ):
            base = (d * NJ + j) * 800
            for bi, (n0, nw) in enumerate(((0, 400), (400, 400))):
                ps = psum_pool.tile([128, 400], F32, tag=tag, name=f"{tag}_ps")
                for k in range(nk):
                    nc.tensor.matmul(
                        ps[0:128, 0:nw],
                        rhs_chunks[k][:, 128 * j:128 * (j + 1)],
                        w_chunks[k][:, d * 800 + n0:d * 800 + n0 + nw],
                        start=(k == 0), stop=(k == nk - 1),
                    )
                dst = XW[0:128, base + n0:base + n0 + nw]
                if bi == 0:
                    nc.vector.tensor_copy(dst, ps[0:128, 0:nw])
                else:
                    nc.scalar.copy(dst, ps[0:128, 0:nw])


def _emit_lstm_layer(nc, tc, XW, u_chunks, a_out, G, hT, hTs, CT, Tif, Tg,
                     TO, TC, hh, Atmp, Btmp, I, n_fill=3, s_len=None):
    """Emit one full BiLSTM layer (both directions, S steps, fully unrolled).

    a_out: per-dir pairs ((A0,A1),(A2,A3)) of SBUF [128,512]/[72,512] bf16
    tiles that receive hT columns; they also serve as the recurrent lhsT.

    G layout: PSUM bank0 holds [i|f] at cols 0:400, bank1 holds [g|o] at
    cols 512:912.  tanh(i,f) only needs bank0, so it starts after half the
    recurrent matmuls; tanh(g) overlaps bank1's matmuls on the PE.

    The xw injection for step s+1 is emitted at the END of step s's block so
    its PSUM WAR dependency (the tanh reads of G) is already satisfied when
    the in-order PE queue reaches it -- no head-of-line stall.  Filler
    matmuls into a scratch PSUM bank keep the PE's HAM activity monitor busy
    so it stays at K=8/8 (2.4 GHz) instead of throttling to half clock.
    """
    if s_len is None:
        s_len = S
    BANKS = ((0, 0, 400), (400, 512, 400))   # (xw col, G col, width)

    def inject(d, t, stop):
        base = (d * NJ + t // TBLK) * 800
        r0 = (t % TBLK) * BL
        for xn0, gn0, nw in BANKS:
            nc.tensor.matmul(
                G[d][0:BL, gn0:gn0 + nw],
                I[:, r0:r0 + BL],
                XW[0:128, base + xn0:base + xn0 + nw],
                start=True, stop=stop,
                skip_group_check=True,
            )

    with tc.psum_pool(name="fill", bufs=1) as FPL:
        fps = FPL.tile([BL, 512], F32, name="fill_ps")
        for d in range(2):
            nc.vector.memset(CT[d][0:BL, 0:200], 0.0)
            inject(d, 0 if d == 0 else s_len - 1, stop=True)

        for s in range(s_len):
            for d in range(2):
                t = s if d == 0 else s_len - 1 - s
                t_prev = t - 1 if d == 0 else t + 1
                t_next = t + 1 if d == 0 else t - 1
                g = G[d]
                # ---- recurrent matmuls, bank-major: bank0 ([i|f]) finishes
                # first so tanh(i,f) can start while bank1 ([g|o]) runs
                if s > 0:
                    for xn0, gn0, nw in BANKS:
                        for k in range(2):
                            lhsT = hTs[d][0:UK[k], BL * k:BL * (k + 1)]
                            nc.tensor.matmul(
                                g[0:BL, gn0:gn0 + nw], lhsT,
                                u_chunks[k][:, d * 800 + xn0:d * 800 + xn0 + nw],
                                start=False, stop=(k == 1),
                                skip_group_check=True,
                            )
                for _ in range(n_fill):
                    nc.tensor.matmul(fps[0:BL, 0:256], I[:, 0:BL],
                                     XW[0:128, 0:256],
                                     start=True, stop=True,
                                     skip_group_check=True)
                # ---- activations + cell update
                nc.scalar.activation(Tif[d][0:BL, 0:400], g[0:BL, 0:400],
                                     AF.Tanh)
                nc.vector.scalar_tensor_tensor(
                    Atmp[d][0:BL, 0:200], Tif[d][0:BL, 200:400], 1.0,
                    CT[d][0:BL, 0:200], ALU.add, ALU.mult)
                nc.scalar.activation(Tg[d][0:BL, 0:200], g[0:BL, 512:712],
                                     AF.Tanh)
                nc.vector.scalar_tensor_tensor(
                    Btmp[d][0:BL, 0:200], Tif[d][0:BL, 0:200], 1.0,
                    Tg[d][0:BL, 0:200], ALU.add, ALU.mult)
                nc.vector.scalar_tensor_tensor(
                    CT[d][0:BL, 0:200], Atmp[d][0:BL, 0:200], 0.5,
                    Btmp[d][0:BL, 0:200], ALU.mult, ALU.add)
                # tanh(o/2): off the h->h critical chain (before tanh_c)
                nc.scalar.activation(TO[d][0:BL, 0:200], g[0:BL, 712:912],
                                     AF.Tanh, scale=0.5)
                nc.scalar.activation(TC[d][0:BL, 0:200], CT[d][0:BL, 0:200],
                                     AF.Tanh, scale=0.5)
                nc.vector.scalar_tensor_tensor(
                    hh[d][0:BL, 0:200], TO[d][0:BL, 0:200], 1.0,
                    TC[d][0:BL, 0:200], ALU.add, ALU.mult)
                # ---- transpose ht into the accumulator tensors; the two
                # PSUM->SBUF casts go to different engines so they overlap
                nc.tensor.matmul(hT[d][0:128, 0:BL], hh[d][0:BL, 0:128],
                                 I[0:BL, 0:BL],
                                 start=True, stop=False, skip_group_check=True)
                nc.tensor.matmul(hT[d][0:72, BL:2 * BL], hh[d][0:BL, 128:200],
                                 I[0:BL, 0:BL],
                                 start=False, stop=True, skip_group_check=True)
                nc.vector.tensor_copy(hTs[d][0:128, 0:2 * BL],
                                      hT[d][0:128, 0:2 * BL])
                nc.gpsimd.tensor_copy(a_out[d][0][:, BL * t:BL * (t + 1)],
                                      hTs[d][0:128, 0:BL])
                nc.gpsimd.tensor_copy(a_out[d][1][:, BL * t:BL * (t + 1)],
                                      hTs[d][0:72, BL:2 * BL])
                # ---- next step's xw injection
                if s + 1 < s_len:
                    inject(d, t_next, stop=False)


def _fix_pe_multiwaits(nc):
    """Walrus codegen rejects PE Matmult with >1 sync wait (LDWEIGHTS struct
    has a single wait slot).  Hoist extra waits onto PE NoOps inserted just
    before the offending matmult."""
    total = 0
    for fnc in nc.m.functions:
        for blk in fnc.blocks:
            lst = blk.instructions
            out = []
            for ins in lst:
                si = ins.sync_info
                cap = 1
                if si is not None and len(si.on_wait) > cap:
                    si_cls = type(si)
                    extra = list(si.on_wait[:-cap])
                    keep = si.on_wait[-cap]
                    for j, w in enumerate(extra):
                        nop = mybir.InstNoOp(
                            name=f"{ins.name}_wnop{j}", ins=[], outs=[])
                        nop.engine = ins.engine
                        nop.sync_info = si_cls(on_wait=[w], on_update=[])
                        out.append(nop)
                    ins.sync_info = si_cls(on_wait=[keep],
                                           on_update=list(si.on_update))
                    total += 1
                out.append(ins)
            blk.instructions = out
    return total


def build_program(fix_multiwait=True):
    nc = bass.Bass("TRN2", target_bir_lowering=False, debug=False)

    # ---- DRAM tensors (per-core inputs; SPMD same program)
    d_xt = nc.dram_tensor("xt", [XROWS, SB], BF16, kind="ExternalInput").ap()
    d_w0 = nc.dram_tensor("w0", [XROWS, 1600], BF16, kind="ExternalInput").ap()
    d_u0 = nc.dram_tensor("u0", [200, 1600], BF16, kind="ExternalInput").ap()
    d_w1 = nc.dram_tensor("w1", [X1ROWS, 1600], BF16, kind="ExternalInput").ap()
    d_u1 = nc.dram_tensor("u1", [200, 1600], BF16, kind="ExternalInput").ap()
    d_ow = nc.dram_tensor("ow", [X1ROWS, TAGS], BF16, kind="ExternalInput").ap()
    d_i128 = nc.dram_tensor("i128", [128, 128], BF16, kind="ExternalInput").ap()
    d_out = nc.dram_tensor("out", [BL, S, TAGS], F32, kind="ExternalOutput").ap()

    with tile.TileContext(nc) as tc:
        with tc.sbuf_pool(name="persist", bufs=1) as SP, \
             tc.psum_pool(name="gates", bufs=1) as GP:
            # persistent SBUF tiles
            I = SP.tile([128, 128], BF16, name="ident")
            u0c = [SP.tile([UK[k], 1600], BF16, name=f"u0c{k}") for k in range(2)]
            u1c = [SP.tile([UK[k], 1600], BF16, name=f"u1c{k}") for k in range(2)]
            XW0 = SP.tile([128, 2 * NJ * 800], BF16, name="XW0")
            XW1 = SP.tile([128, 2 * NJ * 800], BF16, name="XW1")
            # layer-0 / layer-1 hidden-state accumulators ("A buffers"):
            # [128,512] and [72,512] per direction; rows = h-units (transposed)
            A0 = [[SP.tile([X1K[2 * d + k], SB], BF16, name=f"A0_{d}_{k}")
                   for k in range(2)] for d in range(2)]
            A1 = [[SP.tile([X1K[2 * d + k], SB], BF16, name=f"A1_{d}_{k}")
                   for k in range(2)] for d in range(2)]
            ones = SP.tile([1, SB], BF16, name="ones")
            owc = []
            row = 0
            for k, kk in enumerate(X1K):
                owc.append(SP.tile([kk, TAGS], BF16, name=f"owc{k}"))
                row += kk
            # small per-direction work tiles (bf16 elementwise;
            # numerics validated at rel err 2.2e-3 vs the 2e-2 tolerance)
            Tif = [SP.tile([BL, 400], BF16, name=f"Tif{d}") for d in range(2)]
            Tg = [SP.tile([BL, 200], BF16, name=f"Tg{d}") for d in range(2)]
            TO = [SP.tile([BL, 200], BF16, name=f"TO{d}") for d in range(2)]
            hTs = [SP.tile([128, 2 * BL], BF16, name=f"hTs{d}")
                   for d in range(2)]
            TC = [SP.tile([BL, 200], BF16, name=f"TC{d}") for d in range(2)]
            hh = [SP.tile([BL, 200], BF16, name=f"hh{d}") for d in range(2)]
            Atmp = [SP.tile([BL, 200], BF16, name=f"At{d}") for d in range(2)]
            Btmp = [SP.tile([BL, 200], BF16, name=f"Bt{d}") for d in range(2)]
            CT = [SP.tile([BL, 200], BF16, name=f"CT{d}") for d in range(2)]
            # PSUM gate tiles + hT transpose tiles
            G = [GP.tile([BL, GW], F32, name=f"G{d}") for d in range(2)]
            hT = [GP.tile([128, 2 * BL], F32, name=f"hT{d}") for d in range(2)]

            # ---- load persistent weights
            nc.sync.dma_start(I, d_i128)
            for k in range(2):
                nc.sync.dma_start(u0c[k], d_u0[(0, 128)[k]:(128, 200)[k], :])
                nc.sync.dma_start(u1c[k], d_u1[(0, 128)[k]:(128, 200)[k], :])
            nc.vector.memset(ones[0:1, 0:SB], 1.0)
            row = 0
            for k, kk in enumerate(X1K):
                nc.sync.dma_start(owc[k], d_ow[row:row + kk, :])
                row += kk

            # w1 chunks are loaded up-front so the layer-0 -> layer-1
            # transition never waits on DMA
            w1c = [SP.tile([X1K[k], 1600], BF16, name=f"w1c{k}")
                   for k in range(5)]
            row = 0
            for k, kk in enumerate(X1K):
                nc.sync.dma_start(w1c[k], d_w1[row:row + kk, :])
                row += kk

            # ---- phase 2: xw0 precompute
            with tc.sbuf_pool(name="ph2", bufs=1) as P2S, \
                 tc.psum_pool(name="ph2p", bufs=2) as P2P:
                xTc = [P2S.tile([XK[k], SB], BF16, name=f"xTc{k}") for k in range(3)]
                w0c = [P2S.tile([XK[k], 1600], BF16, name=f"w0c{k}") for k in range(3)]
                row = 0
                for k, kk in enumerate(XK):
                    nc.sync.dma_start(xTc[k], d_xt[row:row + kk, :])
                    nc.sync.dma_start(w0c[k], d_w0[row:row + kk, :])
                    row += kk
                _emit_xw_precompute(nc, tc, P2P, w0c, xTc, XW0, tag="xw0p")

            # ---- phase 3: layer-0 recurrence
            _emit_lstm_layer(nc, tc, XW0, u0c, A0, G, hT, hTs, CT, Tif, Tg,
                             TO, TC, hh, Atmp, Btmp, I)

            # ---- phase 4: xw1 precompute (input = A0 buffers + ones)
            with tc.psum_pool(name="ph4p", bufs=2) as P4P:
                rhs1 = [A0[0][0], A0[0][1], A0[1][0], A0[1][1], ones]
                _emit_xw_precompute(nc, tc, P4P, w1c, rhs1, XW1, tag="xw1p")

            # ---- phase 5: layer-1 recurrence
            _emit_lstm_layer(nc, tc, XW1, u1c, A1, G, hT, hTs, CT, Tif, Tg,
                             TO, TC, hh, Atmp, Btmp, I)

            # ---- phase 6: output head
            with tc.sbuf_pool(name="fin", bufs=2) as FS, \
                 tc.psum_pool(name="finp", bufs=2) as FP:
                lhs_chunks = [A1[0][0], A1[0][1], A1[1][0], A1[1][1], ones]
                out_r = d_out.rearrange("b t e -> t b e")
                mt = min(128, SB)
                for m in range(SB // mt):
                    po = FP.tile([mt, TAGS], F32, tag="po", name="po")
                    for k in range(5):
                        nc.tensor.matmul(
                            po[0:mt, 0:TAGS],
                            lhs_chunks[k][:, mt * m:mt * (m + 1)],
                            owc[k],
                            start=(k == 0), stop=(k == 4),
                        )
                    so = FS.tile([mt, TAGS], F32, tag="so", name="so")
                    nc.scalar.activation(so[0:mt, 0:TAGS], po[0:mt, 0:TAGS],
                                         AF.Sigmoid)
                    nc.sync.dma_start(out_r[(mt // BL) * m:(mt // BL) * (m + 1), :, :],
                                      so[0:mt, 0:TAGS])

    if fix_multiwait:
        _fix_pe_multiwaits(nc)
    return nc


_CACHE = {}


def kernel(**inputs):
    inputs = {k: np.asarray(v) for k, v in inputs.items()}
    words = inputs["words"]

    shared = _prep_weights(
        inputs["emb_table"], inputs["lstm_Wih0"], inputs["lstm_Whh0"],
        inputs["lstm_b0"], inputs["lstm_Wih1"], inputs["lstm_Whh1"],
        inputs["lstm_b1"], inputs["out_w"], inputs["out_b"])

    in_maps = []
    for c in range(NCORES):
        xt = _prep_xt(inputs["emb_table"], words[c * BL:(c + 1) * BL])
        in_maps.append({"xt": xt, **shared})

    if "nc" not in _CACHE:
        _CACHE["nc"] = build_program()
    nc = _CACHE["nc"]

    res = bass_utils.run_bass_kernel_spmd(
        nc, in_maps, core_ids=list(range(NCORES)),
        trace=_CACHE.get("trace", False),
        tmpdir=_CACHE.get("tmpdir"))
    _CACHE["last_exec_ns"] = res.exec_time_ns
    _CACHE["last_res"] = res

    out = np.concatenate([res.results[c]["out"] for c in range(NCORES)], axis=0)
    return out.astype(np.float32)
